# revision 20
# baseline (speedup 1.0000x reference)
"""Trainium2 Bass kernel for CosmicNetGNN (NNConv message passing).

Strategy: shard nodes into 8 contiguous dst-bands (2048 nodes/core); host
sorts edges by dst so each core owns all in-edges of its band.  Per layer,
each core builds per-edge outer products z[e,(i,k)] = h[src[e],i]*e2[e,k]
split across the vector + gpsimd engines, then scatters them TRANSPOSED:
S^T[(i,k), n] accumulates in PSUM via matmuls with z chunks as the
stationary operand and a host-precomputed, deg_inv-scaled indicator as the
moving operand.  That layout feeds the reorganized ew3 matrix M[(i,k),o]
directly (no PE transposes, no wide S staging), and the root-weight term
accumulates into the same PSUM tile, so LayerNorm reads one finished
pre-activation.  Everything hot is bf16 (PE runs 4x faster than fp32;
PSUM accumulation stays fp32); the edge MLP output e2 and the own-band h
stay resident in SBUF; per-node-tile h[src] gathers are batched into one
indirect DMA (994ns fixed cost per op otherwise); LayerNorm & leaky-relu
run mostly on the scalar engine using one activation table
(copy/square/sqrt/parametric_relu).
"""
import sys
sys.path.insert(0, '/opt/trn_rl_repo')
import numpy as np
import ml_dtypes
from concourse import bass, mybir, tile, bacc
from concourse import bass_utils
from concourse.masks import make_identity

N, E, B = 16384, 65536, 32
D_IN, ED, H, L = 4, 5, 64, 3
NEG = 0.1
EPS = 1e-5
NC = 8
BAND = N // NC       # 2048
NT = BAND // 128     # 16 node tiles per core
F32 = mybir.dt.float32
BF16 = mybir.dt.bfloat16
I32 = mybir.dt.int32
BFNP = ml_dtypes.bfloat16
SPLIT = 64           # z columns computed on DVE; rest on gpsimd (real-HW
                     # gpsimd broadcast-mult is ~10x slower than modeled, so
                     # DVE takes all of z and gpsimd gets the small LN ops)


def _host_prep(inputs):
    x = np.asarray(inputs['x'], np.float32)
    ei = np.asarray(inputs['edge_index']).astype(np.int64)
    ea = np.asarray(inputs['edge_attr'], np.float32)
    batch = np.asarray(inputs['batch']).astype(np.int64)
    src, dst = ei[0], ei[1]

    deg = np.bincount(dst, minlength=N).astype(np.float32)
    deg_inv = (1.0 / np.maximum(deg, 1.0)).astype(np.float32)

    order = np.argsort(dst, kind='stable')
    src_s, dst_s, ea_s = src[order], dst[order], ea[order]
    gt = dst_s // 128
    counts = np.bincount(gt, minlength=N // 128)
    T_et = int(np.ceil(counts.max() / 128))
    EP = NT * T_et * 128
    ET = NT * T_et

    src_pad = np.zeros((NC, EP), np.int32)
    ea_pad = np.zeros((NC, EP, ED), np.float32)
    ind_pack = np.zeros((NC, 128, ET * 128), np.float32)
    starts = np.concatenate([[0], np.cumsum(counts)])
    for c in range(NC):
        for t in range(NT):
            g = c * NT + t
            s, e = starts[g], starts[g + 1]
            cnt = e - s
            o = t * T_et * 128
            src_pad[c, o:o + cnt] = src_s[s:e]
            ea_pad[c, o:o + cnt] = ea_s[s:e]
            pos = np.arange(cnt)
            j_loc = pos // 128
            p = pos % 128
            cols = (t * T_et + j_loc) * 128 + (dst_s[s:e] - g * 128)
            ind_pack[c, p, cols] = deg_inv[dst_s[s:e]]

    eaT1 = np.concatenate([ea_pad.transpose(0, 2, 1),
                           np.ones((NC, 1, EP), np.float32)], axis=1)  # [NC,6,EP]

    cnt_b = np.bincount(batch, minlength=B).astype(np.float32)
    onehot_sc = np.zeros((N, B), np.float32)
    onehot_sc[np.arange(N), batch] = 1.0 / np.maximum(cnt_b, 1.0)[batch]

    ew3 = np.asarray(inputs['ew3'], np.float32)
    eb3 = np.asarray(inputs['eb3'], np.float32)
    NCH = H * H // 128 + 1            # 33 chunks of the [4160,64] M_aug
    M_pack = np.zeros((L, 128, NCH * H), np.float32)
    for l in range(L):
        w = ew3[l].reshape(H, H, H)                   # [i, o, k]
        M_aug = np.zeros((NCH * 128, H), np.float32)
        M_aug[:H * H] = w.transpose(0, 2, 1).reshape(H * H, H)   # [(i,k), o]
        M_aug[H * H:H * H + H] = eb3[l].reshape(H, H)            # [i, o]
        for q in range(NCH):
            M_pack[l, :, q * H:(q + 1) * H] = M_aug[q * 128:(q + 1) * 128]

    def aug(wT, b):  # [K,H']+[H'] -> [K+1,H']
        return np.concatenate([np.asarray(wT, np.float32),
                               np.asarray(b, np.float32)[None, :]], 0)

    def bf(a):
        return np.asarray(a, np.float32).astype(BFNP)

    xT1 = np.concatenate([x.T, np.ones((1, N), np.float32)], 0)  # [5,N]
    # h-table rows are stored half-major (all cores' first band halves,
    # then all second halves) so each half-AllGather writes a contiguous
    # range.  pr[n] = h_tab row of node n.
    ar = np.arange(N)
    c_of = ar // BAND
    loc = ar % BAND
    HB = BAND // 2
    pr = np.where(loc < HB, c_of * HB + loc,
                  NC * HB + c_of * HB + (loc - HB)).astype(np.int64)
    inv = np.empty(N, np.int64)
    inv[pr] = ar
    xT1_band = xT1  # original node order, for per-core own-band projection
    xT1 = xT1[:, inv]
    onehot_sc = onehot_sc[inv]
    src_perm = pr[src_pad.astype(np.int64)].astype(np.int32)  # [NC,EP]
    host = dict(
        T_et=T_et, EP=EP,
        src_idx=np.stack([src_perm[c].reshape(-1, 128).T
                          for c in range(NC)]),                         # [NC,128,ET]
        ind_pack=bf(ind_pack),
        eaT1=bf(eaT1),
        onehot_sc=bf(onehot_sc),
        M_pack=bf(M_pack),
        xT1=bf(xT1),
        x_bandT=bf(np.stack([xT1_band[:, c * BAND:(c + 1) * BAND]
                             for c in range(NC)])),                     # [NC,5,BAND]
        win_rhs=bf(aug(np.asarray(inputs['W_in']).T, inputs['b_in'])),  # [5,64]
        ew1_rhs=bf(np.stack([aug(np.asarray(inputs['ew1'][l]).T, inputs['eb1'][l])
                             for l in range(L)])),                      # [L,6,64]
        ew2_rhs=bf(np.stack([aug(np.asarray(inputs['ew2'][l]).T, inputs['eb2'][l])
                             for l in range(L)])),                      # [L,65,64]
        root_rhs=bf(np.stack([aug(np.asarray(inputs['root_w'][l]).T, inputs['root_b'][l])
                              for l in range(L)])),                     # [L,65,64]
        ln_g=np.broadcast_to(np.asarray(inputs['ln_g'], np.float32)[:, None, :],
                             (L, 128, H)).copy(),                       # [L,128,64]
        ln_b=np.broadcast_to(np.asarray(inputs['ln_b'], np.float32)[:, None, :],
                             (L, 128, H)).copy(),
        pw1_rhs=aug(np.asarray(inputs['pw1']).T, inputs['pb1']),        # [65,64]
        pw2_rhs=aug(np.asarray(inputs['pw2']).T, inputs['pb2']),        # [65,32]
        pw3_rhs=aug(np.asarray(inputs['pw3']).T, inputs['pb3']),        # [33,1]
    )
    return host


def _leaky(nc, pool, out_ap, in_ap, shape, dt=F32):
    """out = max(in, 0.1*in) — safe leaky relu via 2 DVE ops."""
    tmp = pool.tile(list(shape), dt, tag='lk_tmp')
    nc.vector.tensor_scalar_mul(tmp[:], in_ap, NEG)
    nc.vector.tensor_tensor(out=out_ap, in0=tmp[:], in1=in_ap,
                            op=mybir.AluOpType.max)


def _build(T_et, EP):
    ET = NT * T_et  # edge tiles per core
    NCH = H * H // 128 + 1
    PRELU = mybir.ActivationFunctionType.Prelu
    COPY = mybir.ActivationFunctionType.Copy
    IDENT = mybir.ActivationFunctionType.Identity
    SQUARE = mybir.ActivationFunctionType.Square
    SQRT = mybir.ActivationFunctionType.Sqrt
    nc = bacc.Bacc('TRN2', target_bir_lowering=False, debug=False,
                   num_devices=NC)

    def din(name, shape, dt=F32):
        return nc.dram_tensor(name, list(shape), dt, kind='ExternalInput')

    t_xT1 = din('xT1', [D_IN + 1, N], BF16)
    t_xb = din('x_bandT', [D_IN + 1, BAND], BF16)
    t_win = din('win_rhs', [D_IN + 1, H], BF16)
    t_eaT1 = din('eaT1', [ED + 1, EP], BF16)
    t_ew1 = din('ew1_rhs', [L, ED + 1, H], BF16)
    t_ew2 = din('ew2_rhs', [L, H + 1, H], BF16)
    t_root = din('root_rhs', [L, H + 1, H], BF16)
    t_M = din('M_pack', [L, 128, NCH * H], BF16)
    t_lng = din('ln_g', [L, 128, H])
    t_lnb = din('ln_b', [L, 128, H])
    t_srci = din('src_idx', [128, ET], I32)
    t_ind = din('ind_pack', [128, ET * 128], BF16)
    t_oh = din('onehot_sc', [N, B], BF16)
    t_pw1 = din('pw1_rhs', [H + 1, H])
    t_pw2 = din('pw2_rhs', [H + 1, B])
    t_pw3 = din('pw3_rhs', [B + 1, 1])
    t_out = nc.dram_tensor('pred', [1, B], F32, kind='ExternalOutput')

    # ST-chunk waves: chunks 0..31 are z columns, chunk 32 is the h_src part
    waves = [list(range(w * 4, min(w * 4 + 4, NCH))) for w in range((NCH + 3) // 4)]

    with tile.TileContext(nc) as tc:
        with (tc.tile_pool(name='const', bufs=1) as cp,
              tc.tile_pool(name='hob', bufs=NT + 2) as hob,
              tc.tile_pool(name='dram', bufs=1, space='DRAM') as dram):
            ident = cp.tile([128, 128], BF16)
            make_identity(nc, ident[:])
            srci_sb = cp.tile([128, ET], I32)
            nc.sync.dma_start(out=srci_sb[:], in_=t_srci[:, :])
            ind_sb = cp.tile([128, ET * 128], BF16)
            nc.sync.dma_start(out=ind_sb[:], in_=t_ind[:, :])
            M_sb = [cp.tile([128, NCH * H], BF16, name=f'Msb{l}', tag=f'M{l}') for l in range(L)]
            for l in range(L):
                nc.sync.dma_start(out=M_sb[l][:], in_=t_M[l, :, :])
            root_sb = [cp.tile([H + 1, H], BF16, name=f'rtsb{l}', tag=f'rt{l}') for l in range(L)]
            lng_sb = [cp.tile([128, H], F32, name=f'lgsb{l}', tag=f'lg{l}') for l in range(L)]
            lnb_sb = [cp.tile([128, H], F32, name=f'lbsb{l}', tag=f'lb{l}') for l in range(L)]
            for l in range(L):
                nc.sync.dma_start(out=root_sb[l][:], in_=t_root[l, :, :])
                nc.sync.dma_start(out=lng_sb[l][:], in_=t_lng[l, :, :])
                nc.sync.dma_start(out=lnb_sb[l][:], in_=t_lnb[l, :, :])
            # e2 for all layers stays resident in SBUF: [128, ET*H] bf16 each
            e2_sb = [cp.tile([128, ET * H], BF16, name=f'e2sb{l}', tag=f'e2{l}')
                     for l in range(L)]

            h_tab = [dram.tile([N, H], BF16, name=f'htab{i}', tag=f'h{i}') for i in range(L + 1)]
            band_d = [dram.tile([BAND, H], BF16, name=f'bandd{l}', tag=f'bd{l}') for l in range(L)]

            # ---- stage 0: input projection h0 = leaky(x @ W_in.T + b) ----
            hob_prev = []
            with (tc.tile_pool(name='s0', bufs=3) as s0,
                  tc.tile_pool(name='s0c', bufs=1) as s0c,
                  tc.tile_pool(name='s0p', bufs=3, space='PSUM') as s0p):
                xT_sb = s0c.tile([D_IN + 1, N], BF16)
                nc.sync.dma_start(out=xT_sb[:], in_=t_xT1[:, :])
                xb_sb = s0c.tile([D_IN + 1, BAND], BF16)
                nc.sync.dma_start(out=xb_sb[:], in_=t_xb[:, :])
                win_sb = s0c.tile([D_IN + 1, H], BF16)
                nc.sync.dma_start(out=win_sb[:], in_=t_win[:, :])
                for g in range(N // 128):
                    ps = s0p.tile([128, H], F32, tag='p')
                    nc.tensor.matmul(out=ps[:], lhsT=xT_sb[:, g * 128:(g + 1) * 128],
                                     rhs=win_sb[:], start=True, stop=True)
                    h0t = s0.tile([128, H], BF16, tag='h0')
                    nc.scalar.activation(out=h0t[:], in_=ps[:], func=PRELU,
                                         alpha=NEG)
                    nc.sync.dma_start(out=h_tab[0][g * 128:(g + 1) * 128, :],
                                      in_=h0t[:])
                # own band h0, kept resident in SBUF
                for nt in range(NT):
                    ps = s0p.tile([128, H], F32, tag='p')
                    nc.tensor.matmul(out=ps[:],
                                     lhsT=xb_sb[:, nt * 128:(nt + 1) * 128],
                                     rhs=win_sb[:], start=True, stop=True)
                    ho = hob.tile([128, H], BF16, tag='hob')
                    nc.scalar.activation(out=ho[:], in_=ps[:], func=PRELU,
                                         alpha=NEG)
                    hob_prev.append(ho)

            # ---- stage 0b: edge MLP e2 for all layers (SBUF-resident out) ----
            with (tc.tile_pool(name='em', bufs=3) as em,
                  tc.tile_pool(name='emc', bufs=1) as emc,
                  tc.tile_pool(name='emp', bufs=3, space='PSUM') as emp):
                ea_sb = emc.tile([ED + 1, EP], BF16)
                nc.sync.dma_start(out=ea_sb[:], in_=t_eaT1[:, :])
                e1_sb = emc.tile([H + 1, EP], BF16)
                nc.vector.memset(e1_sb[H:H + 1, :], 1.0)
                for l in range(L):
                    w1 = em.tile([ED + 1, H], BF16, tag='w1')
                    nc.sync.dma_start(out=w1[:], in_=t_ew1[l, :, :])
                    w2 = em.tile([H + 1, H], BF16, tag='w2')
                    nc.sync.dma_start(out=w2[:], in_=t_ew2[l, :, :])
                    for q in range(EP // 512):
                        ps1 = emp.tile([H, 512], F32, tag='p1')
                        nc.tensor.matmul(out=ps1[:],
                                         lhsT=w1[:],
                                         rhs=ea_sb[:, q * 512:(q + 1) * 512],
                                         start=True, stop=True)
                        nc.scalar.activation(
                            out=e1_sb[0:H, q * 512:(q + 1) * 512], in_=ps1[:],
                            func=PRELU, alpha=NEG)
                    for et in range(ET):
                        ps2 = emp.tile([128, H], F32, tag='p2')
                        nc.tensor.matmul(out=ps2[:],
                                         lhsT=e1_sb[:, et * 128:(et + 1) * 128],
                                         rhs=w2[:], start=True, stop=True)
                        nc.scalar.activation(
                            out=e2_sb[l][:, et * H:(et + 1) * H], in_=ps2[:],
                            func=PRELU, alpha=NEG)

            # ---- layers ----
            with (tc.tile_pool(name='zz', bufs=T_et + 3) as zp,
                  tc.tile_pool(name='ly', bufs=4) as ly,
                  tc.tile_pool(name='lyn', bufs=4) as lyn,
                  tc.tile_pool(name='stb', bufs=4) as stp,
                  tc.tile_pool(name='pS', bufs=3, space='PSUM') as pS,
                  tc.tile_pool(name='ptp', bufs=2, space='PSUM') as ptp,
                  tc.tile_pool(name='pag', bufs=2, space='PSUM') as pag):
                for l in range(L):
                    hin = h_tab[l]
                    hob_cur = []
                    for nt in range(NT):
                        # one batched gather for all T_et edge tiles
                        hs_all = ly.tile([128, T_et * H], BF16, tag='hs')
                        nc.gpsimd.indirect_dma_start(
                            out=hs_all[:], out_offset=None,
                            in_=hin[:, :],
                            in_offset=bass.IndirectOffsetOnAxis(
                                ap=srci_sb[:, nt * T_et:(nt + 1) * T_et], axis=0))
                        z_t = []
                        for j in range(T_et):
                            hs = hs_all[:, j * H:(j + 1) * H]
                            z = zp.tile([128, H * H], BF16, tag='z')
                            zv = z[:].rearrange('p (i k) -> p i k', i=H)
                            e2v = e2_sb[l][:, (nt * T_et + j) * H:
                                           (nt * T_et + j + 1) * H]
                            sp = min(SPLIT, H)
                            nc.vector.tensor_tensor(
                                out=zv[:, 0:sp, :],
                                in0=e2v[:, None, :].to_broadcast([128, sp, H]),
                                in1=hs[:, 0:sp].to_broadcast([128, sp, H]),
                                op=mybir.AluOpType.mult)
                            if sp < H:
                                nc.gpsimd.tensor_tensor(
                                    out=zv[:, sp:H, :],
                                    in0=e2v[:, None, :].to_broadcast(
                                        [128, H - sp, H]),
                                    in1=hs[:, sp:H].to_broadcast(
                                        [128, H - sp, H]),
                                    op=mybir.AluOpType.mult)
                            z_t.append(z)

                        # S^T accumulation in PSUM + M matmuls into agg
                        agg = pag.tile([128, H], F32, tag='agg')
                        for wave in waves:
                            stps = pS.tile([128, 512], F32, tag='sp')
                            for jj, qq in enumerate(wave):
                                kk = 128 if qq < NCH - 1 else H
                                for j in range(T_et):
                                    if qq < NCH - 1:
                                        lhsT = z_t[j][:, qq * 128:qq * 128 + kk]
                                    else:
                                        lhsT = hs_all[:, j * H:(j + 1) * H]
                                    nc.tensor.matmul(
                                        out=stps[0:kk, jj * 128:jj * 128 + 128],
                                        lhsT=lhsT,
                                        rhs=ind_sb[:, (nt * T_et + j) * 128:
                                                   (nt * T_et + j + 1) * 128],
                                        start=(j == 0), stop=(j == T_et - 1))
                            stb = stp.tile([128, 512], BF16, tag='st')
                            nc.scalar.copy(out=stb[:, 0:len(wave) * 128],
                                           in_=stps[:, 0:len(wave) * 128])
                            for jj, qq in enumerate(wave):
                                kk = 128 if qq < NCH - 1 else H
                                nc.tensor.matmul(
                                    out=agg[:],
                                    lhsT=stb[0:kk, jj * 128:jj * 128 + 128],
                                    rhs=M_sb[l][0:kk, qq * H:(qq + 1) * H],
                                    start=(qq == 0), stop=False)
                        # root term accumulates into the same PSUM tile
                        ho = hob_prev[nt]
                        htp = ptp.tile([128, 128], BF16, tag='tp')
                        nc.tensor.transpose(out=htp[0:H, :], in_=ho[:],
                                            identity=ident[:])
                        hoT = lyn.tile([H + 1, 128], BF16, tag='hoT')
                        nc.scalar.copy(out=hoT[0:H, :], in_=htp[0:H, :])
                        nc.gpsimd.memset(hoT[H:H + 1, :], 1.0)
                        nc.tensor.matmul(out=agg[:], lhsT=hoT[:], rhs=root_sb[l][:],
                                         start=False, stop=True)
                        # LayerNorm + leaky + residual (Act-heavy pipeline)
                        xs = lyn.tile([128, H], F32, tag='xs')
                        mu = lyn.tile([128, 1], F32, tag='mu')
                        nc.scalar.activation(out=xs[:], in_=agg[:], func=COPY,
                                             accum_out=mu[:])
                        nmu = lyn.tile([128, 1], F32, tag='nmu')
                        nc.gpsimd.tensor_scalar_mul(nmu[:], mu[:], -1.0 / H)
                        sq = lyn.tile([128, H], F32, tag='sq')
                        vs = lyn.tile([128, 1], F32, tag='vs')
                        nc.scalar.activation(out=sq[:], in_=xs[:], func=SQUARE,
                                             bias=nmu[:], accum_out=vs[:])
                        vse = lyn.tile([128, 1], F32, tag='vse')
                        nc.gpsimd.tensor_scalar(
                            out=vse[:], in0=vs[:], scalar1=1.0 / H, scalar2=EPS,
                            op0=mybir.AluOpType.mult, op1=mybir.AluOpType.add)
                        sd = lyn.tile([128, 1], F32, tag='sd')
                        nc.scalar.activation(out=sd[:], in_=vse[:], func=SQRT)
                        rs = lyn.tile([128, 1], F32, tag='rs')
                        nc.vector.reciprocal(out=rs[:], in_=sd[:])
                        nmr = lyn.tile([128, 1], F32, tag='nmr')
                        nc.gpsimd.tensor_tensor(out=nmr[:], in0=nmu[:], in1=rs[:],
                                                op=mybir.AluOpType.mult)
                        yv = lyn.tile([128, H], F32, tag='yv')
                        nc.scalar.activation(out=yv[:], in_=xs[:], func=IDENT,
                                             scale=rs[:], bias=nmr[:])
                        nc.gpsimd.tensor_tensor(out=yv[:], in0=yv[:],
                                                in1=lng_sb[l][:],
                                                op=mybir.AluOpType.mult)
                        nc.gpsimd.tensor_tensor(out=yv[:], in0=yv[:],
                                                in1=lnb_sb[l][:],
                                                op=mybir.AluOpType.add)
                        lk = lyn.tile([128, H], F32, tag='lk')
                        nc.scalar.activation(out=lk[:], in_=yv[:], func=PRELU,
                                             alpha=NEG)
                        hb = hob.tile([128, H], BF16, tag='hob')
                        nc.gpsimd.tensor_tensor(out=hb[:], in0=lk[:], in1=ho[:],
                                                op=mybir.AluOpType.add)
                        hob_cur.append(hb)
                        nc.sync.dma_start(
                            out=band_d[l][nt * 128:(nt + 1) * 128, :], in_=hb[:])
                        if nt == NT // 2 - 1:
                            # first half of the band is done: overlap its
                            # AllGather with the second half's compute
                            nc.gpsimd.collective_compute(
                                'AllGather', mybir.AluOpType.bypass,
                                replica_groups=[list(range(NC))],
                                ins=[band_d[l][0:BAND // 2, :].opt()],
                                outs=[h_tab[l + 1][0:N // 2, :].opt()])
                    hob_prev = hob_cur
                    nc.gpsimd.collective_compute(
                        'AllGather', mybir.AluOpType.bypass,
                        replica_groups=[list(range(NC))],
                        ins=[band_d[l][BAND // 2:BAND, :].opt()],
                        outs=[h_tab[l + 1][N // 2:N, :].opt()])

            # ---- pool + head (replicated on all cores) ----
            with (tc.tile_pool(name='hd', bufs=3) as hd,
                  tc.tile_pool(name='hdc', bufs=1) as hdc,
                  tc.tile_pool(name='hdp', bufs=2, space='PSUM') as hdp):
                pool_ps = hdp.tile([H, B], F32, tag='pool')
                for g in range(N // 128):
                    ht = hd.tile([128, H], BF16, tag='ht')
                    nc.sync.dma_start(out=ht[:],
                                      in_=h_tab[L][g * 128:(g + 1) * 128, :])
                    oh = hd.tile([128, B], BF16, tag='oh')
                    nc.sync.dma_start(out=oh[:], in_=t_oh[g * 128:(g + 1) * 128, :])
                    nc.tensor.matmul(out=pool_ps[:], lhsT=ht[:], rhs=oh[:],
                                     start=(g == 0), stop=(g == N // 128 - 1))
                pT = hdc.tile([H + 1, B], F32)
                nc.scalar.copy(out=pT[0:H, :], in_=pool_ps[:])
                nc.vector.memset(pT[H:H + 1, :], 1.0)
                w1 = hdc.tile([H + 1, H], F32)
                nc.sync.dma_start(out=w1[:], in_=t_pw1[:, :])
                w2 = hdc.tile([H + 1, B], F32)
                nc.sync.dma_start(out=w2[:], in_=t_pw2[:, :])
                w3 = hdc.tile([B + 1, 1], F32)
                nc.sync.dma_start(out=w3[:], in_=t_pw3[:, :])
                p1ps = hdp.tile([H, B], F32, tag='p1')
                nc.tensor.matmul(out=p1ps[:], lhsT=w1[:], rhs=pT[:],
                                 start=True, stop=True)
                p1 = hdc.tile([H + 1, B], F32)
                _leaky(nc, hd, p1[0:H, :], p1ps[:], (H, B))
                nc.vector.memset(p1[H:H + 1, :], 1.0)
                p2ps = hdp.tile([B, B], F32, tag='p2')
                nc.tensor.matmul(out=p2ps[:], lhsT=w2[:], rhs=p1[:],
                                 start=True, stop=True)
                p2 = hdc.tile([B + 1, B], F32)
                _leaky(nc, hd, p2[0:B, :], p2ps[:], (B, B))
                nc.vector.memset(p2[B:B + 1, :], 1.0)
                p3ps = hdp.tile([1, B], F32, tag='p3')
                nc.tensor.matmul(out=p3ps[:], lhsT=w3[:], rhs=p2[:],
                                 start=True, stop=True)
                pr = hdc.tile([1, B], F32)
                nc.scalar.copy(out=pr[:], in_=p3ps[:])
                nc.sync.dma_start(out=t_out[:, :], in_=pr[:])

    nc.compile()
    return nc


_CACHE = {}

_SHARED_KEYS = ('xT1', 'win_rhs', 'ew1_rhs', 'ew2_rhs', 'root_rhs', 'M_pack',
                'ln_g', 'ln_b', 'onehot_sc', 'pw1_rhs', 'pw2_rhs', 'pw3_rhs')
_PER_CORE_KEYS = ('eaT1', 'src_idx', 'ind_pack', 'x_bandT')


def kernel(**inputs) -> np.ndarray:
    host = _host_prep(inputs)
    T_et, EP = host['T_et'], host['EP']
    key = (T_et, EP)
    if key not in _CACHE:
        _CACHE[key] = _build(T_et, EP)
    nc = _CACHE[key]

    shared = {k: host[k] for k in _SHARED_KEYS}
    in_maps = []
    for c in range(NC):
        m = dict(shared)
        for k in _PER_CORE_KEYS:
            m[k] = host[k][c]
        in_maps.append({k: np.ascontiguousarray(v) for k, v in m.items()})

    res = bass_utils.run_bass_kernel_spmd(nc, in_maps, core_ids=list(range(NC)))
    return np.asarray(res.results[0]['pred'][0], np.float32)


# revision 21
# speedup vs baseline: 1.0846x; 1.0846x over previous
"""Trainium2 Bass kernel for CosmicNetGNN (NNConv message passing).

Strategy: shard nodes into 8 contiguous dst-bands (2048 nodes/core); host
sorts edges by dst so each core owns all in-edges of its band.  Per layer,
each core builds per-edge outer products z[e,(i,k)] = h[src[e],i]*e2[e,k]
split across the vector + gpsimd engines, then scatters them TRANSPOSED:
S^T[(i,k), n] accumulates in PSUM via matmuls with z chunks as the
stationary operand and a host-precomputed, deg_inv-scaled indicator as the
moving operand.  That layout feeds the reorganized ew3 matrix M[(i,k),o]
directly (no PE transposes, no wide S staging), and the root-weight term
accumulates into the same PSUM tile, so LayerNorm reads one finished
pre-activation.  Everything hot is bf16 (PE runs 4x faster than fp32;
PSUM accumulation stays fp32); the edge MLP output e2 and the own-band h
stay resident in SBUF; per-node-tile h[src] gathers are batched into one
indirect DMA (994ns fixed cost per op otherwise); LayerNorm & leaky-relu
run mostly on the scalar engine using one activation table
(copy/square/sqrt/parametric_relu).
"""
import sys
sys.path.insert(0, '/opt/trn_rl_repo')
import numpy as np
import ml_dtypes
from concourse import bass, mybir, tile, bacc
from concourse import bass_utils
from concourse.masks import make_identity

N, E, B = 16384, 65536, 32
D_IN, ED, H, L = 4, 5, 64, 3
NEG = 0.1
EPS = 1e-5
NC = 8
BAND = N // NC       # 2048
NT = BAND // 128     # 16 node tiles per core
F32 = mybir.dt.float32
BF16 = mybir.dt.bfloat16
I32 = mybir.dt.int32
BFNP = ml_dtypes.bfloat16
SPLIT = 64           # z columns computed on DVE; rest on gpsimd (real-HW
                     # gpsimd broadcast-mult is ~10x slower than modeled, so
                     # DVE takes all of z and gpsimd gets the small LN ops)


def _host_prep(inputs):
    x = np.asarray(inputs['x'], np.float32)
    ei = np.asarray(inputs['edge_index']).astype(np.int64)
    ea = np.asarray(inputs['edge_attr'], np.float32)
    batch = np.asarray(inputs['batch']).astype(np.int64)
    src, dst = ei[0], ei[1]

    deg = np.bincount(dst, minlength=N).astype(np.float32)
    deg_inv = (1.0 / np.maximum(deg, 1.0)).astype(np.float32)

    order = np.argsort(dst, kind='stable')
    src_s, dst_s, ea_s = src[order], dst[order], ea[order]
    gt = dst_s // 128
    counts = np.bincount(gt, minlength=N // 128)
    T_et = int(np.ceil(counts.max() / 128))
    EP = NT * T_et * 128
    ET = NT * T_et

    src_pad = np.zeros((NC, EP), np.int32)
    ea_pad = np.zeros((NC, EP, ED), np.float32)
    ind_pack = np.zeros((NC, 128, ET * 128), np.float32)
    starts = np.concatenate([[0], np.cumsum(counts)])
    for c in range(NC):
        for t in range(NT):
            g = c * NT + t
            s, e = starts[g], starts[g + 1]
            cnt = e - s
            o = t * T_et * 128
            src_pad[c, o:o + cnt] = src_s[s:e]
            ea_pad[c, o:o + cnt] = ea_s[s:e]
            pos = np.arange(cnt)
            j_loc = pos // 128
            p = pos % 128
            cols = (t * T_et + j_loc) * 128 + (dst_s[s:e] - g * 128)
            ind_pack[c, p, cols] = deg_inv[dst_s[s:e]]

    eaT1 = np.concatenate([ea_pad.transpose(0, 2, 1),
                           np.ones((NC, 1, EP), np.float32)], axis=1)  # [NC,6,EP]

    cnt_b = np.bincount(batch, minlength=B).astype(np.float32)
    onehot_sc = np.zeros((N, B), np.float32)
    onehot_sc[np.arange(N), batch] = 1.0 / np.maximum(cnt_b, 1.0)[batch]

    ew3 = np.asarray(inputs['ew3'], np.float32)
    eb3 = np.asarray(inputs['eb3'], np.float32)
    NCH = H * H // 128 + 1            # 33 chunks of the [4160,64] M_aug
    M_pack = np.zeros((L, 128, NCH * H), np.float32)
    for l in range(L):
        w = ew3[l].reshape(H, H, H)                   # [i, o, k]
        M_aug = np.zeros((NCH * 128, H), np.float32)
        M_aug[:H * H] = w.transpose(0, 2, 1).reshape(H * H, H)   # [(i,k), o]
        M_aug[H * H:H * H + H] = eb3[l].reshape(H, H)            # [i, o]
        for q in range(NCH):
            M_pack[l, :, q * H:(q + 1) * H] = M_aug[q * 128:(q + 1) * 128]

    def aug(wT, b):  # [K,H']+[H'] -> [K+1,H']
        return np.concatenate([np.asarray(wT, np.float32),
                               np.asarray(b, np.float32)[None, :]], 0)

    def bf(a):
        return np.asarray(a, np.float32).astype(BFNP)

    xT1 = np.concatenate([x.T, np.ones((1, N), np.float32)], 0)  # [5,N]
    # h-table rows are stored half-major (all cores' first band halves,
    # then all second halves) so each half-AllGather writes a contiguous
    # range.  pr[n] = h_tab row of node n.
    ar = np.arange(N)
    c_of = ar // BAND
    loc = ar % BAND
    HB = BAND // 2
    pr = np.where(loc < HB, c_of * HB + loc,
                  NC * HB + c_of * HB + (loc - HB)).astype(np.int64)
    inv = np.empty(N, np.int64)
    inv[pr] = ar
    xT1_band = xT1  # original node order, for per-core own-band projection
    xT1 = xT1[:, inv]
    onehot_sc = onehot_sc[inv]
    src_perm = pr[src_pad.astype(np.int64)].astype(np.int32)  # [NC,EP]
    host = dict(
        T_et=T_et, EP=EP,
        src_idx=np.stack([src_perm[c].reshape(-1, 128).T
                          for c in range(NC)]),                         # [NC,128,ET]
        ind_pack=bf(ind_pack),
        eaT1=bf(eaT1),
        onehot_sc=bf(onehot_sc),
        M_pack=bf(M_pack),
        xT1=bf(xT1),
        x_bandT=bf(np.stack([xT1_band[:, c * BAND:(c + 1) * BAND]
                             for c in range(NC)])),                     # [NC,5,BAND]
        win_rhs=bf(aug(np.asarray(inputs['W_in']).T, inputs['b_in'])),  # [5,64]
        ew1_rhs=bf(np.stack([aug(np.asarray(inputs['ew1'][l]).T, inputs['eb1'][l])
                             for l in range(L)])),                      # [L,6,64]
        ew2_rhs=bf(np.stack([aug(np.asarray(inputs['ew2'][l]).T, inputs['eb2'][l])
                             for l in range(L)])),                      # [L,65,64]
        root_rhs=bf(np.stack([aug(np.asarray(inputs['root_w'][l]).T, inputs['root_b'][l])
                              for l in range(L)])),                     # [L,65,64]
        ln_g=np.broadcast_to(np.asarray(inputs['ln_g'], np.float32)[:, None, :],
                             (L, 128, H)).copy(),                       # [L,128,64]
        ln_b=np.broadcast_to(np.asarray(inputs['ln_b'], np.float32)[:, None, :],
                             (L, 128, H)).copy(),
        pw1_rhs=aug(np.asarray(inputs['pw1']).T, inputs['pb1']),        # [65,64]
        pw2_rhs=aug(np.asarray(inputs['pw2']).T, inputs['pb2']),        # [65,32]
        pw3_rhs=aug(np.asarray(inputs['pw3']).T, inputs['pb3']),        # [33,1]
    )
    return host


def _leaky(nc, pool, out_ap, in_ap, shape, dt=F32):
    """out = max(in, 0.1*in) — safe leaky relu via 2 DVE ops."""
    tmp = pool.tile(list(shape), dt, tag='lk_tmp')
    nc.vector.tensor_scalar_mul(tmp[:], in_ap, NEG)
    nc.vector.tensor_tensor(out=out_ap, in0=tmp[:], in1=in_ap,
                            op=mybir.AluOpType.max)


def _build(T_et, EP):
    ET = NT * T_et  # edge tiles per core
    NCH = H * H // 128 + 1
    PRELU = mybir.ActivationFunctionType.Prelu
    COPY = mybir.ActivationFunctionType.Copy
    IDENT = mybir.ActivationFunctionType.Identity
    SQUARE = mybir.ActivationFunctionType.Square
    SQRT = mybir.ActivationFunctionType.Sqrt
    nc = bacc.Bacc('TRN2', target_bir_lowering=False, debug=False,
                   num_devices=NC)

    def din(name, shape, dt=F32):
        return nc.dram_tensor(name, list(shape), dt, kind='ExternalInput')

    t_xT1 = din('xT1', [D_IN + 1, N], BF16)
    t_xb = din('x_bandT', [D_IN + 1, BAND], BF16)
    t_win = din('win_rhs', [D_IN + 1, H], BF16)
    t_eaT1 = din('eaT1', [ED + 1, EP], BF16)
    t_ew1 = din('ew1_rhs', [L, ED + 1, H], BF16)
    t_ew2 = din('ew2_rhs', [L, H + 1, H], BF16)
    t_root = din('root_rhs', [L, H + 1, H], BF16)
    t_M = din('M_pack', [L, 128, NCH * H], BF16)
    t_lng = din('ln_g', [L, 128, H])
    t_lnb = din('ln_b', [L, 128, H])
    t_srci = din('src_idx', [128, ET], I32)
    t_ind = din('ind_pack', [128, ET * 128], BF16)
    t_oh = din('onehot_sc', [N, B], BF16)
    t_pw1 = din('pw1_rhs', [H + 1, H])
    t_pw2 = din('pw2_rhs', [H + 1, B])
    t_pw3 = din('pw3_rhs', [B + 1, 1])
    t_out = nc.dram_tensor('pred', [1, B], F32, kind='ExternalOutput')

    # ST-chunk waves: chunks 0..31 are z columns, chunk 32 is the h_src part
    waves = [list(range(w * 4, min(w * 4 + 4, NCH))) for w in range((NCH + 3) // 4)]

    with tile.TileContext(nc) as tc:
        with (tc.tile_pool(name='const', bufs=1) as cp,
              tc.tile_pool(name='hob', bufs=NT + 2) as hob,
              tc.tile_pool(name='dram', bufs=1, space='DRAM') as dram):
            ident = cp.tile([128, 128], BF16)
            make_identity(nc, ident[:])
            srci_sb = cp.tile([128, ET], I32)
            nc.sync.dma_start(out=srci_sb[:], in_=t_srci[:, :])
            ind_sb = cp.tile([128, ET * 128], BF16)
            nc.sync.dma_start(out=ind_sb[:], in_=t_ind[:, :])
            M_sb = [cp.tile([128, NCH * H], BF16, name=f'Msb{l}', tag=f'M{l}') for l in range(L)]
            for l in range(L):
                nc.sync.dma_start(out=M_sb[l][:], in_=t_M[l, :, :])
            root_sb = [cp.tile([H + 1, H], BF16, name=f'rtsb{l}', tag=f'rt{l}') for l in range(L)]
            lng_sb = [cp.tile([128, H], F32, name=f'lgsb{l}', tag=f'lg{l}') for l in range(L)]
            lnb_sb = [cp.tile([128, H], F32, name=f'lbsb{l}', tag=f'lb{l}') for l in range(L)]
            for l in range(L):
                nc.sync.dma_start(out=root_sb[l][:], in_=t_root[l, :, :])
                nc.sync.dma_start(out=lng_sb[l][:], in_=t_lng[l, :, :])
                nc.sync.dma_start(out=lnb_sb[l][:], in_=t_lnb[l, :, :])
            # e2 for all layers stays resident in SBUF: [128, ET*H] bf16 each
            e2_sb = [cp.tile([128, ET * H], BF16, name=f'e2sb{l}', tag=f'e2{l}')
                     for l in range(L)]

            h_tab = [dram.tile([N, H], BF16, name=f'htab{i}', tag=f'h{i}') for i in range(L + 1)]
            band_d = [dram.tile([BAND, H], BF16, name=f'bandd{l}', tag=f'bd{l}') for l in range(L)]

            # ---- stage 0: input projection h0 = leaky(x @ W_in.T + b) ----
            hob_prev = []
            with (tc.tile_pool(name='s0', bufs=3) as s0,
                  tc.tile_pool(name='s0c', bufs=1) as s0c,
                  tc.tile_pool(name='s0p', bufs=3, space='PSUM') as s0p):
                xT_sb = s0c.tile([D_IN + 1, N], BF16)
                nc.sync.dma_start(out=xT_sb[:], in_=t_xT1[:, :])
                xb_sb = s0c.tile([D_IN + 1, BAND], BF16)
                nc.sync.dma_start(out=xb_sb[:], in_=t_xb[:, :])
                win_sb = s0c.tile([D_IN + 1, H], BF16)
                nc.sync.dma_start(out=win_sb[:], in_=t_win[:, :])
                for g in range(N // 128):
                    ps = s0p.tile([128, H], F32, tag='p')
                    nc.tensor.matmul(out=ps[:], lhsT=xT_sb[:, g * 128:(g + 1) * 128],
                                     rhs=win_sb[:], start=True, stop=True)
                    h0t = s0.tile([128, H], BF16, tag='h0')
                    nc.scalar.activation(out=h0t[:], in_=ps[:], func=PRELU,
                                         alpha=NEG)
                    nc.sync.dma_start(out=h_tab[0][g * 128:(g + 1) * 128, :],
                                      in_=h0t[:])
                # own band h0, kept resident in SBUF
                for nt in range(NT):
                    ps = s0p.tile([128, H], F32, tag='p')
                    nc.tensor.matmul(out=ps[:],
                                     lhsT=xb_sb[:, nt * 128:(nt + 1) * 128],
                                     rhs=win_sb[:], start=True, stop=True)
                    ho = hob.tile([128, H], BF16, tag='hob')
                    nc.scalar.activation(out=ho[:], in_=ps[:], func=PRELU,
                                         alpha=NEG)
                    hob_prev.append(ho)

            # ---- stage 0b: edge MLP e2 for all layers (SBUF-resident out) ----
            with (tc.tile_pool(name='em', bufs=3) as em,
                  tc.tile_pool(name='emc', bufs=1) as emc,
                  tc.tile_pool(name='emp', bufs=3, space='PSUM') as emp):
                ea_sb = emc.tile([ED + 1, EP], BF16)
                nc.sync.dma_start(out=ea_sb[:], in_=t_eaT1[:, :])
                e1_sb = emc.tile([H + 1, EP], BF16)
                nc.vector.memset(e1_sb[H:H + 1, :], 1.0)
                for l in range(L):
                    w1 = em.tile([ED + 1, H], BF16, tag='w1')
                    nc.sync.dma_start(out=w1[:], in_=t_ew1[l, :, :])
                    w2 = em.tile([H + 1, H], BF16, tag='w2')
                    nc.sync.dma_start(out=w2[:], in_=t_ew2[l, :, :])
                    for q in range(EP // 512):
                        ps1 = emp.tile([H, 512], F32, tag='p1')
                        nc.tensor.matmul(out=ps1[:],
                                         lhsT=w1[:],
                                         rhs=ea_sb[:, q * 512:(q + 1) * 512],
                                         start=True, stop=True)
                        nc.scalar.activation(
                            out=e1_sb[0:H, q * 512:(q + 1) * 512], in_=ps1[:],
                            func=PRELU, alpha=NEG)
                    for et in range(ET):
                        ps2 = emp.tile([128, H], F32, tag='p2')
                        nc.tensor.matmul(out=ps2[:],
                                         lhsT=e1_sb[:, et * 128:(et + 1) * 128],
                                         rhs=w2[:], start=True, stop=True)
                        nc.scalar.activation(
                            out=e2_sb[l][:, et * H:(et + 1) * H], in_=ps2[:],
                            func=PRELU, alpha=NEG)

            # ---- layers ----
            with (tc.tile_pool(name='zz', bufs=T_et + 3) as zp,
                  tc.tile_pool(name='ly', bufs=4) as ly,
                  tc.tile_pool(name='lyn', bufs=4) as lyn,
                  tc.tile_pool(name='stb', bufs=4) as stp,
                  tc.tile_pool(name='pS', bufs=3, space='PSUM') as pS,
                  tc.tile_pool(name='ptp', bufs=2, space='PSUM') as ptp,
                  tc.tile_pool(name='pag', bufs=2, space='PSUM') as pag):
                for l in range(L):
                    hin = h_tab[l]
                    hob_cur = []
                    for nt in range(NT):
                        # one batched gather for all T_et edge tiles
                        hs_all = ly.tile([128, T_et * H], BF16, tag='hs')
                        nc.gpsimd.indirect_dma_start(
                            out=hs_all[:], out_offset=None,
                            in_=hin[:, :],
                            in_offset=bass.IndirectOffsetOnAxis(
                                ap=srci_sb[:, nt * T_et:(nt + 1) * T_et], axis=0))
                        z_t = []
                        for j in range(T_et):
                            hs = hs_all[:, j * H:(j + 1) * H]
                            z = zp.tile([128, H * H], BF16, tag='z')
                            zv = z[:].rearrange('p (i k) -> p i k', i=H)
                            e2v = e2_sb[l][:, (nt * T_et + j) * H:
                                           (nt * T_et + j + 1) * H]
                            sp = min(SPLIT, H)
                            nc.vector.tensor_tensor(
                                out=zv[:, 0:sp, :],
                                in0=e2v[:, None, :].to_broadcast([128, sp, H]),
                                in1=hs[:, 0:sp].to_broadcast([128, sp, H]),
                                op=mybir.AluOpType.mult)
                            if sp < H:
                                nc.gpsimd.tensor_tensor(
                                    out=zv[:, sp:H, :],
                                    in0=e2v[:, None, :].to_broadcast(
                                        [128, H - sp, H]),
                                    in1=hs[:, sp:H].to_broadcast(
                                        [128, H - sp, H]),
                                    op=mybir.AluOpType.mult)
                            z_t.append(z)

                        # S^T accumulation in PSUM + M matmuls into agg
                        agg = pag.tile([128, H], F32, tag='agg')
                        for wave in waves:
                            stps = pS.tile([128, 512], F32, tag='sp')
                            for jj, qq in enumerate(wave):
                                kk = 128 if qq < NCH - 1 else H
                                for j in range(T_et):
                                    if qq < NCH - 1:
                                        lhsT = z_t[j][:, qq * 128:qq * 128 + kk]
                                    else:
                                        lhsT = hs_all[:, j * H:(j + 1) * H]
                                    nc.tensor.matmul(
                                        out=stps[0:kk, jj * 128:jj * 128 + 128],
                                        lhsT=lhsT,
                                        rhs=ind_sb[:, (nt * T_et + j) * 128:
                                                   (nt * T_et + j + 1) * 128],
                                        start=(j == 0), stop=(j == T_et - 1))
                            stb = stp.tile([128, 512], BF16, tag='st')
                            nc.scalar.copy(out=stb[:, 0:len(wave) * 128],
                                           in_=stps[:, 0:len(wave) * 128])
                            for jj, qq in enumerate(wave):
                                kk = 128 if qq < NCH - 1 else H
                                nc.tensor.matmul(
                                    out=agg[:],
                                    lhsT=stb[0:kk, jj * 128:jj * 128 + 128],
                                    rhs=M_sb[l][0:kk, qq * H:(qq + 1) * H],
                                    start=(qq == 0), stop=False)
                        # root term accumulates into the same PSUM tile
                        ho = hob_prev[nt]
                        htp = ptp.tile([128, 128], BF16, tag='tp')
                        nc.tensor.transpose(out=htp[0:H, :], in_=ho[:],
                                            identity=ident[:])
                        hoT = lyn.tile([H + 1, 128], BF16, tag='hoT')
                        nc.scalar.copy(out=hoT[0:H, :], in_=htp[0:H, :])
                        nc.vector.memset(hoT[H:H + 1, :], 1.0)
                        nc.tensor.matmul(out=agg[:], lhsT=hoT[:], rhs=root_sb[l][:],
                                         start=False, stop=True)
                        # LayerNorm + leaky + residual (Act-heavy pipeline)
                        xs = lyn.tile([128, H], F32, tag='xs')
                        mu = lyn.tile([128, 1], F32, tag='mu')
                        nc.scalar.activation(out=xs[:], in_=agg[:], func=COPY,
                                             accum_out=mu[:])
                        nmu = lyn.tile([128, 1], F32, tag='nmu')
                        nc.vector.tensor_scalar_mul(nmu[:], mu[:], -1.0 / H)
                        sq = lyn.tile([128, H], F32, tag='sq')
                        vs = lyn.tile([128, 1], F32, tag='vs')
                        nc.scalar.activation(out=sq[:], in_=xs[:], func=SQUARE,
                                             bias=nmu[:], accum_out=vs[:])
                        vse = lyn.tile([128, 1], F32, tag='vse')
                        nc.vector.tensor_scalar(
                            out=vse[:], in0=vs[:], scalar1=1.0 / H, scalar2=EPS,
                            op0=mybir.AluOpType.mult, op1=mybir.AluOpType.add)
                        sd = lyn.tile([128, 1], F32, tag='sd')
                        nc.scalar.activation(out=sd[:], in_=vse[:], func=SQRT)
                        rs = lyn.tile([128, 1], F32, tag='rs')
                        nc.vector.reciprocal(out=rs[:], in_=sd[:])
                        nmr = lyn.tile([128, 1], F32, tag='nmr')
                        nc.vector.tensor_tensor(out=nmr[:], in0=nmu[:], in1=rs[:],
                                                op=mybir.AluOpType.mult)
                        yv = lyn.tile([128, H], F32, tag='yv')
                        nc.scalar.activation(out=yv[:], in_=xs[:], func=IDENT,
                                             scale=rs[:], bias=nmr[:])
                        nc.vector.tensor_tensor(out=yv[:], in0=yv[:],
                                                in1=lng_sb[l][:],
                                                op=mybir.AluOpType.mult)
                        nc.vector.tensor_tensor(out=yv[:], in0=yv[:],
                                                in1=lnb_sb[l][:],
                                                op=mybir.AluOpType.add)
                        lk = lyn.tile([128, H], F32, tag='lk')
                        nc.scalar.activation(out=lk[:], in_=yv[:], func=PRELU,
                                             alpha=NEG)
                        hb = hob.tile([128, H], BF16, tag='hob')
                        nc.vector.tensor_tensor(out=hb[:], in0=lk[:], in1=ho[:],
                                                op=mybir.AluOpType.add)
                        hob_cur.append(hb)
                        nc.sync.dma_start(
                            out=band_d[l][nt * 128:(nt + 1) * 128, :], in_=hb[:])
                        if nt == NT // 2 - 1:
                            # first half of the band is done: overlap its
                            # AllGather with the second half's compute
                            nc.gpsimd.collective_compute(
                                'AllGather', mybir.AluOpType.bypass,
                                replica_groups=[list(range(NC))],
                                ins=[band_d[l][0:BAND // 2, :].opt()],
                                outs=[h_tab[l + 1][0:N // 2, :].opt()])
                    hob_prev = hob_cur
                    nc.gpsimd.collective_compute(
                        'AllGather', mybir.AluOpType.bypass,
                        replica_groups=[list(range(NC))],
                        ins=[band_d[l][BAND // 2:BAND, :].opt()],
                        outs=[h_tab[l + 1][N // 2:N, :].opt()])

            # ---- pool + head (replicated on all cores) ----
            with (tc.tile_pool(name='hd', bufs=3) as hd,
                  tc.tile_pool(name='hdc', bufs=1) as hdc,
                  tc.tile_pool(name='hdp', bufs=2, space='PSUM') as hdp):
                pool_ps = hdp.tile([H, B], F32, tag='pool')
                for g in range(N // 128):
                    ht = hd.tile([128, H], BF16, tag='ht')
                    nc.sync.dma_start(out=ht[:],
                                      in_=h_tab[L][g * 128:(g + 1) * 128, :])
                    oh = hd.tile([128, B], BF16, tag='oh')
                    nc.sync.dma_start(out=oh[:], in_=t_oh[g * 128:(g + 1) * 128, :])
                    nc.tensor.matmul(out=pool_ps[:], lhsT=ht[:], rhs=oh[:],
                                     start=(g == 0), stop=(g == N // 128 - 1))
                pT = hdc.tile([H + 1, B], F32)
                nc.scalar.copy(out=pT[0:H, :], in_=pool_ps[:])
                nc.vector.memset(pT[H:H + 1, :], 1.0)
                w1 = hdc.tile([H + 1, H], F32)
                nc.sync.dma_start(out=w1[:], in_=t_pw1[:, :])
                w2 = hdc.tile([H + 1, B], F32)
                nc.sync.dma_start(out=w2[:], in_=t_pw2[:, :])
                w3 = hdc.tile([B + 1, 1], F32)
                nc.sync.dma_start(out=w3[:], in_=t_pw3[:, :])
                p1ps = hdp.tile([H, B], F32, tag='p1')
                nc.tensor.matmul(out=p1ps[:], lhsT=w1[:], rhs=pT[:],
                                 start=True, stop=True)
                p1 = hdc.tile([H + 1, B], F32)
                _leaky(nc, hd, p1[0:H, :], p1ps[:], (H, B))
                nc.vector.memset(p1[H:H + 1, :], 1.0)
                p2ps = hdp.tile([B, B], F32, tag='p2')
                nc.tensor.matmul(out=p2ps[:], lhsT=w2[:], rhs=p1[:],
                                 start=True, stop=True)
                p2 = hdc.tile([B + 1, B], F32)
                _leaky(nc, hd, p2[0:B, :], p2ps[:], (B, B))
                nc.vector.memset(p2[B:B + 1, :], 1.0)
                p3ps = hdp.tile([1, B], F32, tag='p3')
                nc.tensor.matmul(out=p3ps[:], lhsT=w3[:], rhs=p2[:],
                                 start=True, stop=True)
                pr = hdc.tile([1, B], F32)
                nc.scalar.copy(out=pr[:], in_=p3ps[:])
                nc.sync.dma_start(out=t_out[:, :], in_=pr[:])

    nc.compile()
    return nc


_CACHE = {}

_SHARED_KEYS = ('xT1', 'win_rhs', 'ew1_rhs', 'ew2_rhs', 'root_rhs', 'M_pack',
                'ln_g', 'ln_b', 'onehot_sc', 'pw1_rhs', 'pw2_rhs', 'pw3_rhs')
_PER_CORE_KEYS = ('eaT1', 'src_idx', 'ind_pack', 'x_bandT')


def kernel(**inputs) -> np.ndarray:
    host = _host_prep(inputs)
    T_et, EP = host['T_et'], host['EP']
    key = (T_et, EP)
    if key not in _CACHE:
        _CACHE[key] = _build(T_et, EP)
    nc = _CACHE[key]

    shared = {k: host[k] for k in _SHARED_KEYS}
    in_maps = []
    for c in range(NC):
        m = dict(shared)
        for k in _PER_CORE_KEYS:
            m[k] = host[k][c]
        in_maps.append({k: np.ascontiguousarray(v) for k, v in m.items()})

    res = bass_utils.run_bass_kernel_spmd(nc, in_maps, core_ids=list(range(NC)))
    return np.asarray(res.results[0]['pred'][0], np.float32)


# revision 24
# speedup vs baseline: 1.6792x; 1.5482x over previous
"""Trainium2 Bass kernel for CosmicNetGNN (NNConv message passing).

Strategy: shard nodes into 8 contiguous dst-bands (2048 nodes/core); host
sorts edges by dst so each core owns all in-edges of its band.  Per layer,
each core builds per-edge outer products z[e,(i,k)] = h[src[e],i]*e2[e,k]
split across the vector + gpsimd engines, then scatters them TRANSPOSED:
S^T[(i,k), n] accumulates in PSUM via matmuls with z chunks as the
stationary operand and a host-precomputed, deg_inv-scaled indicator as the
moving operand.  That layout feeds the reorganized ew3 matrix M[(i,k),o]
directly (no PE transposes, no wide S staging), and the root-weight term
accumulates into the same PSUM tile, so LayerNorm reads one finished
pre-activation.  Everything hot is bf16 (PE runs 4x faster than fp32;
PSUM accumulation stays fp32); the edge MLP output e2 and the own-band h
stay resident in SBUF; per-node-tile h[src] gathers are batched into one
indirect DMA (994ns fixed cost per op otherwise); LayerNorm & leaky-relu
run mostly on the scalar engine using one activation table
(copy/square/sqrt/parametric_relu).
"""
import sys
sys.path.insert(0, '/opt/trn_rl_repo')
import numpy as np
import ml_dtypes
from concourse import bass, mybir, tile, bacc
from concourse import bass_utils
from concourse.masks import make_identity

N, E, B = 16384, 65536, 32
D_IN, ED, H, L = 4, 5, 64, 3
NEG = 0.1
EPS = 1e-5
NC = 8
BAND = N // NC       # 2048
NT = BAND // 128     # 16 node tiles per core
F32 = mybir.dt.float32
BF16 = mybir.dt.bfloat16
I32 = mybir.dt.int32
BFNP = ml_dtypes.bfloat16
SPLIT = 64           # z columns computed on DVE; rest on gpsimd (real-HW
                     # gpsimd broadcast-mult is ~10x slower than modeled, so
                     # DVE takes all of z)
SPLIT_AG = True      # split each AllGather into two halves (required: the
                     # h table is stored half-major; also hides half the
                     # collective behind the second half-band's compute)


def _host_prep(inputs):
    x = np.asarray(inputs['x'], np.float32)
    ei = np.asarray(inputs['edge_index']).astype(np.int64)
    ea = np.asarray(inputs['edge_attr'], np.float32)
    batch = np.asarray(inputs['batch']).astype(np.int64)
    src, dst = ei[0], ei[1]

    deg = np.bincount(dst, minlength=N).astype(np.float32)
    deg_inv = (1.0 / np.maximum(deg, 1.0)).astype(np.float32)

    order = np.argsort(dst, kind='stable')
    src_s, dst_s, ea_s = src[order], dst[order], ea[order]
    gt = dst_s // 128
    counts = np.bincount(gt, minlength=N // 128)
    T_et = int(np.ceil(counts.max() / 128))
    EP = NT * T_et * 128
    ET = NT * T_et

    src_pad = np.zeros((NC, EP), np.int32)
    ea_pad = np.zeros((NC, EP, ED), np.float32)
    ind_pack = np.zeros((NC, 128, ET * 128), np.float32)
    starts = np.concatenate([[0], np.cumsum(counts)])
    for c in range(NC):
        for t in range(NT):
            g = c * NT + t
            s, e = starts[g], starts[g + 1]
            cnt = e - s
            o = t * T_et * 128
            src_pad[c, o:o + cnt] = src_s[s:e]
            ea_pad[c, o:o + cnt] = ea_s[s:e]
            pos = np.arange(cnt)
            j_loc = pos // 128
            p = pos % 128
            cols = (t * T_et + j_loc) * 128 + (dst_s[s:e] - g * 128)
            ind_pack[c, p, cols] = deg_inv[dst_s[s:e]]

    eaT1 = np.concatenate([ea_pad.transpose(0, 2, 1),
                           np.ones((NC, 1, EP), np.float32)], axis=1)  # [NC,6,EP]

    cnt_b = np.bincount(batch, minlength=B).astype(np.float32)
    onehot_sc = np.zeros((N, B), np.float32)
    onehot_sc[np.arange(N), batch] = 1.0 / np.maximum(cnt_b, 1.0)[batch]

    ew3 = np.asarray(inputs['ew3'], np.float32)
    eb3 = np.asarray(inputs['eb3'], np.float32)
    NCH = H * H // 128 + 1            # 33 chunks of the [4160,64] M_aug
    M_pack = np.zeros((L, 128, NCH * H), np.float32)
    for l in range(L):
        w = ew3[l].reshape(H, H, H)                   # [i, o, k]
        M_aug = np.zeros((NCH * 128, H), np.float32)
        M_aug[:H * H] = w.transpose(0, 2, 1).reshape(H * H, H)   # [(i,k), o]
        M_aug[H * H:H * H + H] = eb3[l].reshape(H, H)            # [i, o]
        for q in range(NCH):
            M_pack[l, :, q * H:(q + 1) * H] = M_aug[q * 128:(q + 1) * 128]

    def aug(wT, b):  # [K,H']+[H'] -> [K+1,H']
        return np.concatenate([np.asarray(wT, np.float32),
                               np.asarray(b, np.float32)[None, :]], 0)

    def bf(a):
        return np.asarray(a, np.float32).astype(BFNP)

    xT1 = np.concatenate([x.T, np.ones((1, N), np.float32)], 0)  # [5,N]
    # h-table rows are stored half-major (all cores' first band halves,
    # then all second halves) so each half-AllGather writes a contiguous
    # range.  pr[n] = h_tab row of node n.
    ar = np.arange(N)
    c_of = ar // BAND
    loc = ar % BAND
    HB = BAND // 2
    pr = np.where(loc < HB, c_of * HB + loc,
                  NC * HB + c_of * HB + (loc - HB)).astype(np.int64)
    inv = np.empty(N, np.int64)
    inv[pr] = ar
    xT1_band = xT1  # original node order, for per-core own-band projection
    xT1 = xT1[:, inv]
    onehot_sc = onehot_sc[inv]
    src_perm = pr[src_pad.astype(np.int64)].astype(np.int32)  # [NC,EP]
    host = dict(
        T_et=T_et, EP=EP,
        src_idx=np.stack([src_perm[c].reshape(-1, 128).T
                          for c in range(NC)]),                         # [NC,128,ET]
        ind_pack=bf(ind_pack),
        eaT1=bf(eaT1),
        onehot_sc=bf(onehot_sc),
        M_pack=bf(M_pack),
        xT1=bf(xT1),
        x_bandT=bf(np.stack([xT1_band[:, c * BAND:(c + 1) * BAND]
                             for c in range(NC)])),                     # [NC,5,BAND]
        win_rhs=bf(aug(np.asarray(inputs['W_in']).T, inputs['b_in'])),  # [5,64]
        ew1_rhs=bf(np.stack([aug(np.asarray(inputs['ew1'][l]).T, inputs['eb1'][l])
                             for l in range(L)])),                      # [L,6,64]
        ew2_rhs=bf(np.stack([aug(np.asarray(inputs['ew2'][l]).T, inputs['eb2'][l])
                             for l in range(L)])),                      # [L,65,64]
        root_rhs=bf(np.stack([aug(np.asarray(inputs['root_w'][l]).T, inputs['root_b'][l])
                              for l in range(L)])),                     # [L,65,64]
        ln_g=np.broadcast_to(np.asarray(inputs['ln_g'], np.float32)[:, None, :],
                             (L, 128, H)).copy(),                       # [L,128,64]
        ln_b=np.broadcast_to(np.asarray(inputs['ln_b'], np.float32)[:, None, :],
                             (L, 128, H)).copy(),
        pw1_rhs=aug(np.asarray(inputs['pw1']).T, inputs['pb1']),        # [65,64]
        pw2_rhs=aug(np.asarray(inputs['pw2']).T, inputs['pb2']),        # [65,32]
        pw3_rhs=aug(np.asarray(inputs['pw3']).T, inputs['pb3']),        # [33,1]
    )
    return host


def _leaky(nc, pool, out_ap, in_ap, shape, dt=F32):
    """out = max(in, 0.1*in) — safe leaky relu via 2 DVE ops."""
    tmp = pool.tile(list(shape), dt, tag='lk_tmp')
    nc.vector.tensor_scalar_mul(tmp[:], in_ap, NEG)
    nc.vector.tensor_tensor(out=out_ap, in0=tmp[:], in1=in_ap,
                            op=mybir.AluOpType.max)


def _build(T_et, EP):
    ET = NT * T_et  # edge tiles per core
    NCH = H * H // 128 + 1
    PRELU = mybir.ActivationFunctionType.Prelu
    COPY = mybir.ActivationFunctionType.Copy
    IDENT = mybir.ActivationFunctionType.Identity
    SQUARE = mybir.ActivationFunctionType.Square
    SQRT = mybir.ActivationFunctionType.Sqrt
    nc = bacc.Bacc('TRN2', target_bir_lowering=False, debug=False,
                   num_devices=NC)

    def din(name, shape, dt=F32):
        return nc.dram_tensor(name, list(shape), dt, kind='ExternalInput')

    t_xT1 = din('xT1', [D_IN + 1, N], BF16)
    t_xb = din('x_bandT', [D_IN + 1, BAND], BF16)
    t_win = din('win_rhs', [D_IN + 1, H], BF16)
    t_eaT1 = din('eaT1', [ED + 1, EP], BF16)
    t_ew1 = din('ew1_rhs', [L, ED + 1, H], BF16)
    t_ew2 = din('ew2_rhs', [L, H + 1, H], BF16)
    t_root = din('root_rhs', [L, H + 1, H], BF16)
    t_M = din('M_pack', [L, 128, NCH * H], BF16)
    t_lng = din('ln_g', [L, 128, H])
    t_lnb = din('ln_b', [L, 128, H])
    t_srci = din('src_idx', [128, ET], I32)
    t_ind = din('ind_pack', [128, ET * 128], BF16)
    t_oh = din('onehot_sc', [N, B], BF16)
    t_pw1 = din('pw1_rhs', [H + 1, H])
    t_pw2 = din('pw2_rhs', [H + 1, B])
    t_pw3 = din('pw3_rhs', [B + 1, 1])
    t_out = nc.dram_tensor('pred', [1, B], F32, kind='ExternalOutput')

    # ST-chunk waves: chunks 0..31 are z columns, chunk 32 is the h_src part
    waves = [list(range(w * 4, min(w * 4 + 4, NCH))) for w in range((NCH + 3) // 4)]

    with tile.TileContext(nc) as tc:
        with (tc.tile_pool(name='const', bufs=1) as cp,
              tc.tile_pool(name='hob', bufs=NT + 2) as hob,
              tc.tile_pool(name='dram', bufs=1, space='DRAM') as dram):
            ident = cp.tile([128, 128], BF16)
            make_identity(nc, ident[:])
            srci_sb = cp.tile([128, ET], I32)
            nc.sync.dma_start(out=srci_sb[:], in_=t_srci[:, :])
            ind_sb = cp.tile([128, ET * 128], BF16)
            nc.sync.dma_start(out=ind_sb[:], in_=t_ind[:, :])
            M_sb = [cp.tile([128, NCH * H], BF16, name=f'Msb{l}', tag=f'M{l}') for l in range(L)]
            for l in range(L):
                nc.sync.dma_start(out=M_sb[l][:], in_=t_M[l, :, :])
            root_sb = [cp.tile([H + 1, H], BF16, name=f'rtsb{l}', tag=f'rt{l}') for l in range(L)]
            lng_sb = [cp.tile([128, H], F32, name=f'lgsb{l}', tag=f'lg{l}') for l in range(L)]
            lnb_sb = [cp.tile([128, H], F32, name=f'lbsb{l}', tag=f'lb{l}') for l in range(L)]
            for l in range(L):
                nc.sync.dma_start(out=root_sb[l][:], in_=t_root[l, :, :])
                nc.sync.dma_start(out=lng_sb[l][:], in_=t_lng[l, :, :])
                nc.sync.dma_start(out=lnb_sb[l][:], in_=t_lnb[l, :, :])
            # e2 for all layers stays resident in SBUF: [128, ET*H] bf16 each
            e2_sb = [cp.tile([128, ET * H], BF16, name=f'e2sb{l}', tag=f'e2{l}')
                     for l in range(L)]

            h_tab = [dram.tile([N, H], BF16, name=f'htab{i}', tag=f'h{i}') for i in range(L + 1)]
            band_d = [dram.tile([BAND, H], BF16, name=f'bandd{l}', tag=f'bd{l}') for l in range(L)]

            # ---- stage 0: input projection h0 = leaky(x @ W_in.T + b) ----
            hob_prev = []
            with (tc.tile_pool(name='s0', bufs=3) as s0,
                  tc.tile_pool(name='s0c', bufs=1) as s0c,
                  tc.tile_pool(name='s0p', bufs=3, space='PSUM') as s0p):
                xT_sb = s0c.tile([D_IN + 1, N], BF16)
                nc.sync.dma_start(out=xT_sb[:], in_=t_xT1[:, :])
                xb_sb = s0c.tile([D_IN + 1, BAND], BF16)
                nc.sync.dma_start(out=xb_sb[:], in_=t_xb[:, :])
                win_sb = s0c.tile([D_IN + 1, H], BF16)
                nc.sync.dma_start(out=win_sb[:], in_=t_win[:, :])
                for g in range(N // 128):
                    ps = s0p.tile([128, H], F32, tag='p')
                    nc.tensor.matmul(out=ps[:], lhsT=xT_sb[:, g * 128:(g + 1) * 128],
                                     rhs=win_sb[:], start=True, stop=True)
                    h0t = s0.tile([128, H], BF16, tag='h0')
                    nc.scalar.activation(out=h0t[:], in_=ps[:], func=PRELU,
                                         alpha=NEG)
                    nc.sync.dma_start(out=h_tab[0][g * 128:(g + 1) * 128, :],
                                      in_=h0t[:])
                # own band h0, kept resident in SBUF
                for nt in range(NT):
                    ps = s0p.tile([128, H], F32, tag='p')
                    nc.tensor.matmul(out=ps[:],
                                     lhsT=xb_sb[:, nt * 128:(nt + 1) * 128],
                                     rhs=win_sb[:], start=True, stop=True)
                    ho = hob.tile([128, H], BF16, tag='hob')
                    nc.scalar.activation(out=ho[:], in_=ps[:], func=PRELU,
                                         alpha=NEG)
                    hob_prev.append(ho)

            # ---- stage 0b: edge MLP e2 for all layers (SBUF-resident out) ----
            with (tc.tile_pool(name='em', bufs=3) as em,
                  tc.tile_pool(name='emc', bufs=1) as emc,
                  tc.tile_pool(name='emp', bufs=3, space='PSUM') as emp):
                ea_sb = emc.tile([ED + 1, EP], BF16)
                nc.sync.dma_start(out=ea_sb[:], in_=t_eaT1[:, :])
                e1_sb = emc.tile([H + 1, EP], BF16)
                nc.vector.memset(e1_sb[H:H + 1, :], 1.0)
                for l in range(L):
                    w1 = em.tile([ED + 1, H], BF16, tag='w1')
                    nc.sync.dma_start(out=w1[:], in_=t_ew1[l, :, :])
                    w2 = em.tile([H + 1, H], BF16, tag='w2')
                    nc.sync.dma_start(out=w2[:], in_=t_ew2[l, :, :])
                    for q in range(EP // 512):
                        ps1 = emp.tile([H, 512], F32, tag='p1')
                        nc.tensor.matmul(out=ps1[:],
                                         lhsT=w1[:],
                                         rhs=ea_sb[:, q * 512:(q + 1) * 512],
                                         start=True, stop=True)
                        nc.scalar.activation(
                            out=e1_sb[0:H, q * 512:(q + 1) * 512], in_=ps1[:],
                            func=PRELU, alpha=NEG)
                    for et in range(ET):
                        ps2 = emp.tile([128, H], F32, tag='p2')
                        nc.tensor.matmul(out=ps2[:],
                                         lhsT=e1_sb[:, et * 128:(et + 1) * 128],
                                         rhs=w2[:], start=True, stop=True)
                        nc.scalar.activation(
                            out=e2_sb[l][:, et * H:(et + 1) * H], in_=ps2[:],
                            func=PRELU, alpha=NEG)

            # ---- layers ----
            with (tc.tile_pool(name='zz', bufs=T_et + 3) as zp,
                  tc.tile_pool(name='ly', bufs=4) as ly,
                  tc.tile_pool(name='lyn', bufs=4) as lyn,
                  tc.tile_pool(name='stb', bufs=4) as stp,
                  tc.tile_pool(name='pS', bufs=3, space='PSUM') as pS,
                  tc.tile_pool(name='ptp', bufs=2, space='PSUM') as ptp,
                  tc.tile_pool(name='pag', bufs=2, space='PSUM') as pag):
                for l in range(L):
                    hin = h_tab[l]
                    hob_cur = []
                    for nt in range(NT):
                        # one batched gather for all T_et edge tiles
                        hs_all = ly.tile([128, T_et * H], BF16, tag='hs')
                        nc.gpsimd.indirect_dma_start(
                            out=hs_all[:], out_offset=None,
                            in_=hin[:, :],
                            in_offset=bass.IndirectOffsetOnAxis(
                                ap=srci_sb[:, nt * T_et:(nt + 1) * T_et], axis=0))
                        z_t = []
                        for j in range(T_et):
                            hs = hs_all[:, j * H:(j + 1) * H]
                            z = zp.tile([128, H * H], BF16, tag='z')
                            zv = z[:].rearrange('p (i k) -> p i k', i=H)
                            e2v = e2_sb[l][:, (nt * T_et + j) * H:
                                           (nt * T_et + j + 1) * H]
                            sp = min(SPLIT, H)
                            nc.vector.tensor_tensor(
                                out=zv[:, 0:sp, :],
                                in0=e2v[:, None, :].to_broadcast([128, sp, H]),
                                in1=hs[:, 0:sp].to_broadcast([128, sp, H]),
                                op=mybir.AluOpType.mult)
                            if sp < H:
                                nc.gpsimd.tensor_tensor(
                                    out=zv[:, sp:H, :],
                                    in0=e2v[:, None, :].to_broadcast(
                                        [128, H - sp, H]),
                                    in1=hs[:, sp:H].to_broadcast(
                                        [128, H - sp, H]),
                                    op=mybir.AluOpType.mult)
                            z_t.append(z)

                        # S^T accumulation in PSUM + M matmuls into agg
                        agg = pag.tile([128, H], F32, tag='agg')
                        for wave in waves:
                            stps = pS.tile([128, 512], F32, tag='sp')
                            for jj, qq in enumerate(wave):
                                kk = 128 if qq < NCH - 1 else H
                                for j in range(T_et):
                                    if qq < NCH - 1:
                                        lhsT = z_t[j][:, qq * 128:qq * 128 + kk]
                                    else:
                                        lhsT = hs_all[:, j * H:(j + 1) * H]
                                    nc.tensor.matmul(
                                        out=stps[0:kk, jj * 128:jj * 128 + 128],
                                        lhsT=lhsT,
                                        rhs=ind_sb[:, (nt * T_et + j) * 128:
                                                   (nt * T_et + j + 1) * 128],
                                        start=(j == 0), stop=(j == T_et - 1))
                            stb = stp.tile([128, 512], BF16, tag='st')
                            nc.scalar.copy(out=stb[:, 0:len(wave) * 128],
                                           in_=stps[:, 0:len(wave) * 128])
                            for jj, qq in enumerate(wave):
                                kk = 128 if qq < NCH - 1 else H
                                nc.tensor.matmul(
                                    out=agg[:],
                                    lhsT=stb[0:kk, jj * 128:jj * 128 + 128],
                                    rhs=M_sb[l][0:kk, qq * H:(qq + 1) * H],
                                    start=(qq == 0), stop=False)
                        # root term accumulates into the same PSUM tile
                        ho = hob_prev[nt]
                        htp = ptp.tile([128, 128], BF16, tag='tp')
                        nc.tensor.transpose(out=htp[0:H, :], in_=ho[:],
                                            identity=ident[:])
                        hoT = lyn.tile([H + 1, 128], BF16, tag='hoT')
                        nc.scalar.copy(out=hoT[0:H, :], in_=htp[0:H, :])
                        nc.vector.memset(hoT[H:H + 1, :], 1.0)
                        nc.tensor.matmul(out=agg[:], lhsT=hoT[:], rhs=root_sb[l][:],
                                         start=False, stop=True)
                        # LayerNorm + leaky + residual (Act-heavy pipeline)
                        xs = lyn.tile([128, H], F32, tag='xs')
                        mu = lyn.tile([128, 1], F32, tag='mu')
                        nc.scalar.activation(out=xs[:], in_=agg[:], func=COPY,
                                             accum_out=mu[:])
                        nmu = lyn.tile([128, 1], F32, tag='nmu')
                        nc.vector.tensor_scalar_mul(nmu[:], mu[:], -1.0 / H)
                        sq = lyn.tile([128, H], F32, tag='sq')
                        vs = lyn.tile([128, 1], F32, tag='vs')
                        nc.scalar.activation(out=sq[:], in_=xs[:], func=SQUARE,
                                             bias=nmu[:], accum_out=vs[:])
                        vse = lyn.tile([128, 1], F32, tag='vse')
                        nc.vector.tensor_scalar(
                            out=vse[:], in0=vs[:], scalar1=1.0 / H, scalar2=EPS,
                            op0=mybir.AluOpType.mult, op1=mybir.AluOpType.add)
                        sd = lyn.tile([128, 1], F32, tag='sd')
                        nc.scalar.activation(out=sd[:], in_=vse[:], func=SQRT)
                        rs = lyn.tile([128, 1], F32, tag='rs')
                        nc.vector.reciprocal(out=rs[:], in_=sd[:])
                        nmr = lyn.tile([128, 1], F32, tag='nmr')
                        nc.vector.tensor_tensor(out=nmr[:], in0=nmu[:], in1=rs[:],
                                                op=mybir.AluOpType.mult)
                        yv = lyn.tile([128, H], F32, tag='yv')
                        nc.scalar.activation(out=yv[:], in_=xs[:], func=IDENT,
                                             scale=rs[:], bias=nmr[:])
                        nc.vector.tensor_tensor(out=yv[:], in0=yv[:],
                                                in1=lng_sb[l][:],
                                                op=mybir.AluOpType.mult)
                        nc.vector.tensor_tensor(out=yv[:], in0=yv[:],
                                                in1=lnb_sb[l][:],
                                                op=mybir.AluOpType.add)
                        lk = lyn.tile([128, H], F32, tag='lk')
                        nc.scalar.activation(out=lk[:], in_=yv[:], func=PRELU,
                                             alpha=NEG)
                        hb = hob.tile([128, H], BF16, tag='hob')
                        nc.vector.tensor_tensor(out=hb[:], in0=lk[:], in1=ho[:],
                                                op=mybir.AluOpType.add)
                        hob_cur.append(hb)
                        nc.sync.dma_start(
                            out=band_d[l][nt * 128:(nt + 1) * 128, :], in_=hb[:])
                        if nt == NT // 2 - 1 and SPLIT_AG:
                            # first half of the band is done: overlap its
                            # AllGather with the second half's compute
                            nc.gpsimd.collective_compute(
                                'AllGather', mybir.AluOpType.bypass,
                                replica_groups=[list(range(NC))],
                                ins=[band_d[l][0:BAND // 2, :].opt()],
                                outs=[h_tab[l + 1][0:N // 2, :].opt()])
                    hob_prev = hob_cur
                    if SPLIT_AG:
                        nc.gpsimd.collective_compute(
                            'AllGather', mybir.AluOpType.bypass,
                            replica_groups=[list(range(NC))],
                            ins=[band_d[l][BAND // 2:BAND, :].opt()],
                            outs=[h_tab[l + 1][N // 2:N, :].opt()])
                    else:
                        nc.gpsimd.collective_compute(
                            'AllGather', mybir.AluOpType.bypass,
                            replica_groups=[list(range(NC))],
                            ins=[band_d[l][:].opt()],
                            outs=[h_tab[l + 1][:, :].opt()])

            # ---- pool + head (replicated on all cores) ----
            with (tc.tile_pool(name='hd', bufs=3) as hd,
                  tc.tile_pool(name='hdc', bufs=1) as hdc,
                  tc.tile_pool(name='hdp', bufs=2, space='PSUM') as hdp):
                pool_ps = hdp.tile([H, B], F32, tag='pool')
                for g in range(N // 128):
                    ht = hd.tile([128, H], BF16, tag='ht')
                    nc.sync.dma_start(out=ht[:],
                                      in_=h_tab[L][g * 128:(g + 1) * 128, :])
                    oh = hd.tile([128, B], BF16, tag='oh')
                    nc.sync.dma_start(out=oh[:], in_=t_oh[g * 128:(g + 1) * 128, :])
                    nc.tensor.matmul(out=pool_ps[:], lhsT=ht[:], rhs=oh[:],
                                     start=(g == 0), stop=(g == N // 128 - 1))
                pT = hdc.tile([H + 1, B], F32)
                nc.scalar.copy(out=pT[0:H, :], in_=pool_ps[:])
                nc.vector.memset(pT[H:H + 1, :], 1.0)
                w1 = hdc.tile([H + 1, H], F32)
                nc.sync.dma_start(out=w1[:], in_=t_pw1[:, :])
                w2 = hdc.tile([H + 1, B], F32)
                nc.sync.dma_start(out=w2[:], in_=t_pw2[:, :])
                w3 = hdc.tile([B + 1, 1], F32)
                nc.sync.dma_start(out=w3[:], in_=t_pw3[:, :])
                p1ps = hdp.tile([H, B], F32, tag='p1')
                nc.tensor.matmul(out=p1ps[:], lhsT=w1[:], rhs=pT[:],
                                 start=True, stop=True)
                p1 = hdc.tile([H + 1, B], F32)
                _leaky(nc, hd, p1[0:H, :], p1ps[:], (H, B))
                nc.vector.memset(p1[H:H + 1, :], 1.0)
                p2ps = hdp.tile([B, B], F32, tag='p2')
                nc.tensor.matmul(out=p2ps[:], lhsT=w2[:], rhs=p1[:],
                                 start=True, stop=True)
                p2 = hdc.tile([B + 1, B], F32)
                _leaky(nc, hd, p2[0:B, :], p2ps[:], (B, B))
                nc.vector.memset(p2[B:B + 1, :], 1.0)
                p3ps = hdp.tile([1, B], F32, tag='p3')
                nc.tensor.matmul(out=p3ps[:], lhsT=w3[:], rhs=p2[:],
                                 start=True, stop=True)
                pr = hdc.tile([1, B], F32)
                nc.scalar.copy(out=pr[:], in_=p3ps[:])
                nc.sync.dma_start(out=t_out[:, :], in_=pr[:])

    nc.compile()
    return nc


_CACHE = {}

_SHARED_KEYS = ('xT1', 'win_rhs', 'ew1_rhs', 'ew2_rhs', 'root_rhs', 'M_pack',
                'ln_g', 'ln_b', 'onehot_sc', 'pw1_rhs', 'pw2_rhs', 'pw3_rhs')
_PER_CORE_KEYS = ('eaT1', 'src_idx', 'ind_pack', 'x_bandT')


def kernel(**inputs) -> np.ndarray:
    host = _host_prep(inputs)
    T_et, EP = host['T_et'], host['EP']
    key = (T_et, EP)
    if key not in _CACHE:
        _CACHE[key] = _build(T_et, EP)
    nc = _CACHE[key]

    shared = {k: host[k] for k in _SHARED_KEYS}
    in_maps = []
    for c in range(NC):
        m = dict(shared)
        for k in _PER_CORE_KEYS:
            m[k] = host[k][c]
        in_maps.append({k: np.ascontiguousarray(v) for k, v in m.items()})

    res = bass_utils.run_bass_kernel_spmd(nc, in_maps, core_ids=list(range(NC)))
    return np.asarray(res.results[0]['pred'][0], np.float32)
